# revision 14
# baseline (speedup 1.0000x reference)
"""Masked nearest-neighbor (AnchorTs2Vec e_an) Trainium2 kernel, v9.

Problem: for e_actv [8192, 256] f32 and host ids [8192], compute
    d2[i,j] = |e_i|^2 + |e_j|^2 - 2 e_i.e_j
    idx[i]  = argmin_{j: host_j != host_i, j != i} d2[i,j]
    e_an    = e_actv[idx]
Returns (e_actv, e_ap, e_an) like the reference.

Device computes a CANDIDATE MASK against a global threshold TAU on an
fp8 DoubleRow approximation of d2 (see v8 notes); host exact-evaluates
marked pairs and rescues uncertified rows. v9 restructures the data
layout to kill the v8 DMA bottleneck (input was 3.58 MB/core of
per-unit duplicated slices; output DMA tail was ~12 us):

CIRCULANT ROW-SLAB LAYOUT. Core k owns rows R_k = [1024k, 1024k+1024)
and computes the (symmetric-unique) pairs of R_k against columns
  C_k = [own block | k+1 | k+2 | k+3 | Z1 | Z2]  (5120 cols)
where Z1/Z2 are complementary 512-col halves of block k+4 chosen so
the 4 quadrants of each d=4 block pair are covered exactly once, and
the diagonal block is upper-triangular via static mask-column offsets.
Units slice two persistent SBUF tiles (L = own rows as lhsT, R = cols
as rhs), so input DMA is the unique data only (~1.6 MB/core), loaded
diag-cols-first so unit 0 starts ~1.5 us in. Per-row thresholds ride
as 64 tail bytes of L (both signs: is_ge for DVE, Sign-bias for ACT).
"""

import numpy as np
import ml_dtypes

import concourse.tile as tile
from concourse import bacc, mybir
from concourse.bass_utils import run_bass_kernel_spmd

N, D = 8192, 256
N_CORES = 8
P = 128
BLK = 1024                  # row/col block per core
RW = 5120                   # rhs unique-column width per core
LW = 1024                   # lhsT unique-row width per core
LT = 32                     # fp8 tail cols of L carrying thresholds
TAU_D2 = 452.0              # global mark threshold on d2
EPS_D2 = 23.0               # device error bound (d2 units)
CERT_D2 = TAU_D2 - 2.0 * EPS_D2

f8 = ml_dtypes.float8_e4m3

# Row-slice structure: slice rs (0..7) covers rows 1024k + rs*128 +
# [0:128). Its 4608 columns (in slice-local space) are [diag 1024 |
# k+1 1024 | k+2 1024 | k+3 1024 | Z 512] where Z = Z1 for rs<4 (A
# rows) and Z2 for rs>=4 (B rows). Three 1536-wide PSUM groups per
# slice (9 N=512 matmuls sharing one lhsT); the diag mask-off skips
# already-covered lower-triangle columns of group 0.
NG = 3
GW = 1536
SW = NG * GW                 # 4608 slice-local columns


def _slice_off(rs):
    return rs * P if rs < 4 else 512 + (rs - 4) * P


def _r_index(rs, c):
    """Map slice-local column to index into the R tile (5120 wide)."""
    return c if c < 4096 else c + (0 if rs < 4 else 512)


# Greedy DVE/ACT assignment balancing measured per-op cost
# (v11 HW trace: DVE ~(208+w)/0.96 ns, ACT ~(312+w)/1.2 ns busy).
def _mask_engines():
    eng = {}
    load = {"dve": 0.0, "act": 1283.0}
    for rs in range(8):
        for g in range(NG):
            we = GW - (_slice_off(rs) if g == 0 else 0)
            c_dve = (208 + we) / 0.96
            c_act = (312 + we) / 1.2
            pick = "dve" if load["dve"] + c_dve <= load["act"] + c_act \
                else "act"
            load[pick] += c_dve if pick == "dve" else c_act
            eng[(rs, g)] = pick
    return eng


MASK_ENG = _mask_engines()

_compiled = None


def _build():
    nc = bacc.Bacc("TRN2", target_bir_lowering=False, debug=False,
                   num_devices=N_CORES)
    data_L = nc.dram_tensor("data_L", [P, 2, LW + LT], mybir.dt.float8e4,
                            kind="ExternalInput").ap()
    data_Rd = nc.dram_tensor("data_Rd", [P, 2, 1024], mybir.dt.float8e4,
                             kind="ExternalInput").ap()
    data_Ro = nc.dram_tensor("data_Ro", [P, 2, RW - 1024],
                             mybir.dt.float8e4,
                             kind="ExternalInput").ap()
    out_mask = nc.dram_tensor("out_mask", [8, P, SW], mybir.dt.uint8,
                              kind="ExternalOutput").ap()

    with tile.TileContext(nc) as tc:
        with tc.tile_pool(name="in", bufs=1) as ip, \
             tc.tile_pool(name="mp", bufs=5) as mp, \
             tc.tile_pool(name="ps", bufs=2, space="PSUM") as ppA, \
             tc.tile_pool(name="pw", bufs=1, space="PSUM") as ppW:
            tL = ip.tile([P, 2, LW + LT], mybir.dt.float8e4, tag="L")
            tRd = ip.tile([P, 2, 1024], mybir.dt.float8e4, tag="Rd")
            tRo = ip.tile([P, 2, RW - 1024], mybir.dt.float8e4, tag="Ro")
            # smallest-first: u0's first weights land earliest
            nc.sync.dma_start(tL[:, :, 0:256], data_L[:, :, 0:256])
            nc.sync.dma_start(tRd, data_Rd)
            nc.sync.dma_start(tL[:, :, 256:LW + LT],
                              data_L[:, :, 256:LW + LT])
            nc.sync.dma_start(tRo, data_Ro)
            # thresholds: [128, 8] f32 per sign in the L tails
            thr_dve = tL[:, 0, LW:LW + LT].bitcast(mybir.dt.float32)
            thr_act = tL[:, 1, LW:LW + LT].bitcast(mybir.dt.float32)

            # HAM warm-up: keep the PE busy during the input-DMA wait so
            # the clock gate flips to 8/8 before the real stream begins.
            wtile = ip.tile([P, 2, 512], mybir.dt.float8e4, tag="wu")
            nc.gpsimd.memset(wtile, 0.0)
            wps = ppW.tile([P, 1024], mybir.dt.float32, tag="wps")
            for _ in range(8):
                nc.tensor.matmul(
                    wps[:, 0:512], wtile[:, 0:2, 0:P],
                    wtile[:, 0:2, 0:512],
                    start=True, stop=True,
                    perf_mode=mybir.MatmulPerfMode.DoubleRow)

            for rs in range(8):
                mask = mp.tile([P, SW], mybir.dt.uint8, tag="m")
                row0 = rs * P
                lhsT = tL[:, 0:2, row0:row0 + P]
                for g in range(NG):
                    ps = ppA.tile([P, GW], mybir.dt.float32, tag="ps")
                    off = _slice_off(rs) if g == 0 else 0
                    for s in range(GW // 512):
                        c = _r_index(rs, g * GW + s * 512)
                        rsrc, rc = (tRd, c) if c < 1024 else (tRo, c - 1024)
                        nc.tensor.matmul(
                            ps[:, s * 512:(s + 1) * 512],
                            lhsT, rsrc[:, 0:2, rc:rc + 512],
                            start=True, stop=True,
                            perf_mode=mybir.MatmulPerfMode.DoubleRow)
                    mslice = mask[:, g * GW + off:(g + 1) * GW]
                    if MASK_ENG[(rs, g)] == "dve":
                        nc.vector.tensor_scalar(
                            mslice, ps[:, off:GW], thr_dve[:, rs:rs + 1],
                            None, op0=mybir.AluOpType.is_ge)
                    else:
                        nc.scalar.activation(
                            mslice, ps[:, off:GW],
                            mybir.ActivationFunctionType.Sign,
                            bias=thr_act[:, rs:rs + 1], scale=1.0)
                    if g == 1:
                        eng = nc.gpsimd if rs % 2 == 0 else nc.sync
                        eng.dma_start(out_mask[rs, :, 0:2 * GW],
                                      mask[:, 0:2 * GW])
                # final chunk on HWDGE (sync): ~0.6us completion latency
                # vs SWDGE ~2us — the last one is the kernel's tail.
                eng = nc.sync if (rs % 2 == 1 or rs == 6) else nc.gpsimd
                eng.dma_start(out_mask[rs, :, 2 * GW:SW],
                              mask[:, 2 * GW:SW])

    nc.compile()
    return nc


def _cols_order(k):
    """The 5120 global column ids of core k's R tile."""
    parts = [np.arange(k * BLK, (k + 1) * BLK)]
    for d in (1, 2, 3):
        c = ((k + d) % 8) * BLK
        parts.append(np.arange(c, c + BLK))
    m = ((k + 4) % 8) * BLK
    if k < 4:
        z1 = np.arange(m, m + 512)
        z2 = np.arange(m + 512, m + BLK)
    else:
        z1 = np.arange(m + 512, m + BLK)
        z2 = np.arange(m, m + 512)
    parts += [z1, z2]
    return np.concatenate(parts)


def _prep_inputs(e_actv: np.ndarray):
    """Per-core input maps: L (own rows + thr tails), Rd, Ro col tiles.

    Augmented fp8 vectors (K = 256 = 254 data dims + 2 sq slots):
      lhsT rows (i side): [ ek_i (254 dims) ; 1 ; 1 ]
      rhs cols  (j side): [ ek_j (254 dims) ; m1_j ; m2_j ]
    where m1 = fp8(-sq_j/2), m2 = fp8(-sq_j/2 - m1), and ek = e with
    the two smallest-max|e| dims dropped. K index = ck*128 + p.
    """
    e = np.ascontiguousarray(np.asarray(e_actv, dtype=np.float32))
    sq32 = (e * e).sum(1, dtype=np.float32)
    s = sq32.astype(np.float64) / 2.0

    drop = np.argsort(np.abs(e).max(0))[:2]
    keep = np.setdiff1d(np.arange(D), drop)
    ek8T = np.ascontiguousarray(e[:, keep].astype(f8).T)   # [254, 8192]

    m1 = (-s).astype(np.float32).astype(f8)
    m2 = (-s - m1.astype(np.float64)).astype(np.float32).astype(f8)

    aug_l = np.empty((2 * P, N), dtype=f8)   # lhsT side (i): data + 1s
    aug_r = np.empty((2 * P, N), dtype=f8)   # rhs side (j): data + sqs
    aug_l[:254] = ek8T
    aug_l[254] = 1.0
    aug_l[255] = 1.0
    aug_r[:254] = ek8T
    aug_r[254] = m1
    aug_r[255] = m2

    in_maps = []
    for k in range(N_CORES):
        rows = np.arange(k * BLK, (k + 1) * BLK)
        cols = _cols_order(k)
        dL = np.zeros((P, 2, LW + LT), dtype=f8)
        dL[:, 0, :LW] = aug_l[0:P][:, rows]
        dL[:, 1, :LW] = aug_l[P:2 * P][:, rows]
        # thresholds: slice ts covers rows k*1024 + ts*128 + [0:128)
        # DVE: is_ge(ps, t) with t = sq_i/2 - TAU/2
        # ACT: Sign(ps + bias) with bias = -t
        t = (s[rows] - TAU_D2 / 2.0).astype(np.float32).reshape(8, P).T
        dLb = dL.view(np.uint8)
        dLb[:, 0, LW:] = np.ascontiguousarray(t).view(np.uint8)
        dLb[:, 1, LW:] = np.ascontiguousarray(-t).view(np.uint8)
        dR = np.empty((P, 2, RW), dtype=f8)
        dR[:, 0, :] = aug_r[0:P][:, cols]
        dR[:, 1, :] = aug_r[P:2 * P][:, cols]
        in_maps.append({"data_L": dL,
                        "data_Rd": np.ascontiguousarray(dR[:, :, :1024]),
                        "data_Ro": np.ascontiguousarray(dR[:, :, 1024:])})
    return in_maps


def _run(in_maps, trace=False, **kw):
    global _compiled
    if _compiled is None:
        _compiled = _build()
    return run_bass_kernel_spmd(_compiled, in_maps, list(range(N_CORES)),
                                trace=trace, **kw)


def _exact_rows(e, sq32, hostv, rows):
    """Exact fp32 masked argmin for given rows (reference arithmetic)."""
    G = e[rows] @ e.T
    d2 = sq32[rows][:, None] + sq32[None, :] - 2.0 * G
    d2 = np.where(hostv[rows][:, None] == hostv[None, :],
                  np.float32(np.inf), d2)
    return d2.argmin(1)


def kernel(e_actv, e_ap, host):
    e = np.ascontiguousarray(np.asarray(e_actv, dtype=np.float32))
    hostv = np.asarray(host).astype(np.int64)
    in_maps = _prep_inputs(e)
    res = _run(in_maps)

    # Collect marked (i, j) pairs from all cores' row-slice masks.
    ii_l, jj_l = [], []
    for k in range(N_CORES):
        m = res.results[k]["out_mask"]         # [8, 128, 4608] uint8
        cols = _cols_order(k)
        for rs in range(8):
            mu = (m[rs] == 1)
            off = _slice_off(rs)
            if off:
                mu[:, :off] = False            # skipped region: garbage
            pp_, cc = np.nonzero(mu)
            rr = cc + (0 if rs < 4 else (cc >= 4096) * 512)
            ii_l.append(k * BLK + rs * P + pp_)
            jj_l.append(cols[rr])
    ii = np.concatenate(ii_l)
    jj = np.concatenate(jj_l)
    # Drop same-host / self pairs (device doesn't mask them).
    keepp = (hostv[ii] != hostv[jj])
    ii, jj = ii[keepp], jj[keepp]

    # Exact fp32 evaluation of candidates (reference arithmetic), one
    # eval per computed pair; symmetrize afterwards (d2 is symmetric).
    sq32 = (e * e).sum(1, dtype=np.float32)
    g = np.empty(len(ii), dtype=np.float32)
    CH = 2 << 20
    for o in range(0, len(ii), CH):
        sl = slice(o, o + CH)
        g[sl] = np.einsum("nd,nd->n", e[ii[sl]], e[jj[sl]], optimize=True)
    d2c = sq32[ii] + sq32[jj] - 2.0 * np.float32(1.0) * g
    dist = np.sqrt(np.maximum(d2c, 0.0), dtype=np.float32)
    ii, jj = np.concatenate([ii, jj]), np.concatenate([jj, ii])
    dist = np.concatenate([dist, dist])

    # Per-row argmin with first-index tie-break.
    order = np.lexsort((jj, dist, ii))
    oi, oj, od = ii[order], jj[order], dist[order]
    first = np.ones(len(oi), dtype=bool)
    first[1:] = oi[1:] != oi[:-1]
    rows_hit = oi[first]
    idx = np.zeros(N, dtype=np.int64)
    best = np.full(N, np.inf, dtype=np.float64)
    idx[rows_hit] = oj[first]
    best[rows_hit] = od[first].astype(np.float64) ** 2

    # near-tie rows: argmin could be rounding-sensitive -> recompute.
    gap = np.full(N, np.inf)
    pos_first = np.flatnonzero(first)
    pos_second = pos_first + 1
    ok2 = pos_second < len(oi)
    same_row = np.zeros(len(pos_first), dtype=bool)
    same_row[ok2] = oi[pos_second[ok2]] == oi[pos_first[ok2]]
    g2 = np.full(len(pos_first), np.inf)
    g2[same_row] = (od[pos_second[same_row]].astype(np.float64) ** 2
                    - od[pos_first[same_row]].astype(np.float64) ** 2)
    gap[rows_hit] = g2

    rescue = (best > CERT_D2) | (gap < 0.05)
    r_rows = np.flatnonzero(rescue)
    if len(r_rows):
        idx[r_rows] = _exact_rows(e, sq32, hostv, r_rows)

    e_an = np.asarray(e_actv)[idx]
    return (np.asarray(e_actv), np.asarray(e_ap), e_an)


# revision 18
# speedup vs baseline: 1.0633x; 1.0633x over previous
"""Masked nearest-neighbor (AnchorTs2Vec e_an) Trainium2 kernel, v9.

Problem: for e_actv [8192, 256] f32 and host ids [8192], compute
    d2[i,j] = |e_i|^2 + |e_j|^2 - 2 e_i.e_j
    idx[i]  = argmin_{j: host_j != host_i, j != i} d2[i,j]
    e_an    = e_actv[idx]
Returns (e_actv, e_ap, e_an) like the reference.

Device computes a CANDIDATE MASK against a global threshold TAU on an
fp8 DoubleRow approximation of d2 (see v8 notes); host exact-evaluates
marked pairs and rescues uncertified rows. v9 restructures the data
layout to kill the v8 DMA bottleneck (input was 3.58 MB/core of
per-unit duplicated slices; output DMA tail was ~12 us):

CIRCULANT ROW-SLAB LAYOUT. Core k owns rows R_k = [1024k, 1024k+1024)
and computes the (symmetric-unique) pairs of R_k against columns
  C_k = [own block | k+1 | k+2 | k+3 | Z1 | Z2]  (5120 cols)
where Z1/Z2 are complementary 512-col halves of block k+4 chosen so
the 4 quadrants of each d=4 block pair are covered exactly once, and
the diagonal block is upper-triangular via static mask-column offsets.
Units slice two persistent SBUF tiles (L = own rows as lhsT, R = cols
as rhs), so input DMA is the unique data only (~1.6 MB/core), loaded
diag-cols-first so unit 0 starts ~1.5 us in. Per-row thresholds ride
as 64 tail bytes of L (both signs: is_ge for DVE, Sign-bias for ACT).
"""

import numpy as np
import ml_dtypes

import concourse.tile as tile
from concourse import bacc, mybir
from concourse.bass_utils import run_bass_kernel_spmd

N, D = 8192, 256
N_CORES = 8
P = 128
BLK = 1024                  # row/col block per core
RW = 5120                   # rhs unique-column width per core
LW = 1024                   # lhsT unique-row width per core
LT = 32                     # fp8 tail cols of L carrying thresholds
TAU_D2 = 452.0              # global mark threshold on d2
EPS_D2 = 23.0               # device error bound (d2 units)
CERT_D2 = TAU_D2 - 2.0 * EPS_D2

f8 = ml_dtypes.float8_e4m3

# Row-slice structure: slice rs (0..7) covers rows 1024k + rs*128 +
# [0:128). Its 4608 columns (in slice-local space) are [diag 1024 |
# k+1 1024 | k+2 1024 | k+3 1024 | Z 512] where Z = Z1 for rs<4 (A
# rows) and Z2 for rs>=4 (B rows). Three 1536-wide PSUM groups per
# slice (9 N=512 matmuls sharing one lhsT); the diag mask-off skips
# already-covered lower-triangle columns of group 0.
NG = 3
GW = 1536
SW = NG * GW                 # 4608 slice-local columns


def _slice_off(rs):
    return rs * P if rs < 4 else 512 + (rs - 4) * P


def _r_index(rs, c):
    """Map slice-local column to index into the R tile (5120 wide)."""
    return c if c < 4096 else c + (0 if rs < 4 else 512)


# Strict DVE/ACT alternation: with the 2-deep PSUM ring, consecutive
# groups on one engine would serialize tile recycling and stall the PE
# into HAM-cold territory. Alternation keeps both engines continuously
# fed (measured: DVE (208+w)/0.96 ns, ACT (312+w)/1.2 ns per op).
def _mask_engines():
    return {(rs, g): ("dve" if (rs * NG + g) % 2 == 0 else "act")
            for rs in range(8) for g in range(NG)}


MASK_ENG = _mask_engines()

_compiled = None


def _build():
    nc = bacc.Bacc("TRN2", target_bir_lowering=False, debug=False,
                   num_devices=N_CORES)
    data_L = nc.dram_tensor("data_L", [P, 2, LW + LT], mybir.dt.float8e4,
                            kind="ExternalInput").ap()
    data_Rd = nc.dram_tensor("data_Rd", [P, 2, 1024], mybir.dt.float8e4,
                             kind="ExternalInput").ap()
    data_Ro = nc.dram_tensor("data_Ro", [P, 2, RW - 1024],
                             mybir.dt.float8e4,
                             kind="ExternalInput").ap()
    out_mask = nc.dram_tensor("out_mask", [8, P, SW], mybir.dt.uint8,
                              kind="ExternalOutput").ap()

    with tile.TileContext(nc) as tc:
        with tc.tile_pool(name="in", bufs=1) as ip, \
             tc.tile_pool(name="mp", bufs=5) as mp, \
             tc.tile_pool(name="ps", bufs=2, space="PSUM") as ppA, \
             tc.tile_pool(name="pw", bufs=1, space="PSUM") as ppW:
            tL = ip.tile([P, 2, LW + LT], mybir.dt.float8e4, tag="L")
            tRd = ip.tile([P, 2, 1024], mybir.dt.float8e4, tag="Rd")
            tRo = ip.tile([P, 2, RW - 1024], mybir.dt.float8e4, tag="Ro")
            # need-ordered: thresholds + first rows, diag cols, then the
            # big Ro (needed by slice 0 group 0's third matmul), L rest.
            nc.sync.dma_start(tL[:, :, 0:LT + 256], data_L[:, :, 0:LT + 256])
            nc.sync.dma_start(tRd, data_Rd)
            nc.sync.dma_start(tRo, data_Ro)
            nc.sync.dma_start(tL[:, :, LT + 256:LT + LW],
                              data_L[:, :, LT + 256:LT + LW])
            # thresholds: [128, 8] f32 per sign in the L head
            thr_dve = tL[:, 0, 0:LT].bitcast(mybir.dt.float32)
            thr_act = tL[:, 1, 0:LT].bitcast(mybir.dt.float32)

            # HAM warm-up: keep the PE busy during the input-DMA wait so
            # the clock gate flips to 8/8 before the real stream begins.
            wtile = ip.tile([P, 2, 512], mybir.dt.float8e4, tag="wu")
            nc.gpsimd.memset(wtile, 0.0)
            wps = ppW.tile([P, 1024], mybir.dt.float32, tag="wps")
            for _ in range(8):
                nc.tensor.matmul(
                    wps[:, 0:512], wtile[:, 0:2, 0:P],
                    wtile[:, 0:2, 0:512],
                    start=True, stop=True,
                    perf_mode=mybir.MatmulPerfMode.DoubleRow)

            for rs in range(8):
                mask = mp.tile([P, SW], mybir.dt.uint8, tag="m")
                row0 = LT + rs * P
                lhsT = tL[:, 0:2, row0:row0 + P]
                for g in range(NG):
                    ps = ppA.tile([P, GW], mybir.dt.float32, tag="ps")
                    off = _slice_off(rs) if g == 0 else 0
                    for s in range(GW // 512):
                        c = _r_index(rs, g * GW + s * 512)
                        rsrc, rc = (tRd, c) if c < 1024 else (tRo, c - 1024)
                        nc.tensor.matmul(
                            ps[:, s * 512:(s + 1) * 512],
                            lhsT, rsrc[:, 0:2, rc:rc + 512],
                            start=True, stop=True,
                            perf_mode=mybir.MatmulPerfMode.DoubleRow)
                    mslice = mask[:, g * GW + off:(g + 1) * GW]
                    if MASK_ENG[(rs, g)] == "dve":
                        nc.vector.tensor_scalar(
                            mslice, ps[:, off:GW], thr_dve[:, rs:rs + 1],
                            None, op0=mybir.AluOpType.is_ge)
                    else:
                        nc.scalar.activation(
                            mslice, ps[:, off:GW],
                            mybir.ActivationFunctionType.Sign,
                            bias=thr_act[:, rs:rs + 1], scale=1.0)
                    if g == 1:
                        eng = nc.gpsimd if rs % 2 == 0 else nc.sync
                        eng.dma_start(out_mask[rs, :, 0:2 * GW],
                                      mask[:, 0:2 * GW])
                # final chunk on HWDGE (sync): ~0.6us completion latency
                # vs SWDGE ~2us — the last one is the kernel's tail.
                eng = nc.sync if (rs % 2 == 1 or rs == 6) else nc.gpsimd
                eng.dma_start(out_mask[rs, :, 2 * GW:SW],
                              mask[:, 2 * GW:SW])

    nc.compile()
    return nc


def _cols_order(k):
    """The 5120 global column ids of core k's R tile."""
    parts = [np.arange(k * BLK, (k + 1) * BLK)]
    for d in (1, 2, 3):
        c = ((k + d) % 8) * BLK
        parts.append(np.arange(c, c + BLK))
    m = ((k + 4) % 8) * BLK
    if k < 4:
        z1 = np.arange(m, m + 512)
        z2 = np.arange(m + 512, m + BLK)
    else:
        z1 = np.arange(m + 512, m + BLK)
        z2 = np.arange(m, m + 512)
    parts += [z1, z2]
    return np.concatenate(parts)


def _prep_inputs(e_actv: np.ndarray):
    """Per-core input maps: L (own rows + thr tails), Rd, Ro col tiles.

    Augmented fp8 vectors (K = 256 = 254 data dims + 2 sq slots):
      lhsT rows (i side): [ ek_i (254 dims) ; 1 ; 1 ]
      rhs cols  (j side): [ ek_j (254 dims) ; m1_j ; m2_j ]
    where m1 = fp8(-sq_j/2), m2 = fp8(-sq_j/2 - m1), and ek = e with
    the two smallest-max|e| dims dropped. K index = ck*128 + p.
    """
    e = np.ascontiguousarray(np.asarray(e_actv, dtype=np.float32))
    sq32 = (e * e).sum(1, dtype=np.float32)
    s = sq32.astype(np.float64) / 2.0

    drop = np.argsort(np.abs(e).max(0))[:2]
    keep = np.setdiff1d(np.arange(D), drop)
    ek8T = np.ascontiguousarray(e[:, keep].astype(f8).T)   # [254, 8192]

    m1 = (-s).astype(np.float32).astype(f8)
    m2 = (-s - m1.astype(np.float64)).astype(np.float32).astype(f8)

    aug_l = np.empty((2 * P, N), dtype=f8)   # lhsT side (i): data + 1s
    aug_r = np.empty((2 * P, N), dtype=f8)   # rhs side (j): data + sqs
    aug_l[:254] = ek8T
    aug_l[254] = 1.0
    aug_l[255] = 1.0
    aug_r[:254] = ek8T
    aug_r[254] = m1
    aug_r[255] = m2

    in_maps = []
    for k in range(N_CORES):
        rows = np.arange(k * BLK, (k + 1) * BLK)
        cols = _cols_order(k)
        dL = np.zeros((P, 2, LW + LT), dtype=f8)
        dL[:, 0, LT:] = aug_l[0:P][:, rows]
        dL[:, 1, LT:] = aug_l[P:2 * P][:, rows]
        # thresholds: slice rs covers rows k*1024 + rs*128 + [0:128)
        # DVE: is_ge(ps, t) with t = sq_i/2 - TAU/2
        # ACT: Sign(ps + bias) with bias = -t
        t = (s[rows] - TAU_D2 / 2.0).astype(np.float32).reshape(8, P).T
        dLb = dL.view(np.uint8)
        dLb[:, 0, :LT] = np.ascontiguousarray(t).view(np.uint8)
        dLb[:, 1, :LT] = np.ascontiguousarray(-t).view(np.uint8)
        dR = np.empty((P, 2, RW), dtype=f8)
        dR[:, 0, :] = aug_r[0:P][:, cols]
        dR[:, 1, :] = aug_r[P:2 * P][:, cols]
        in_maps.append({"data_L": dL,
                        "data_Rd": np.ascontiguousarray(dR[:, :, :1024]),
                        "data_Ro": np.ascontiguousarray(dR[:, :, 1024:])})
    return in_maps


def _run(in_maps, trace=False, **kw):
    global _compiled
    if _compiled is None:
        _compiled = _build()
    return run_bass_kernel_spmd(_compiled, in_maps, list(range(N_CORES)),
                                trace=trace, **kw)


def _exact_rows(e, sq32, hostv, rows):
    """Exact fp32 masked argmin for given rows (reference arithmetic)."""
    G = e[rows] @ e.T
    d2 = sq32[rows][:, None] + sq32[None, :] - 2.0 * G
    d2 = np.where(hostv[rows][:, None] == hostv[None, :],
                  np.float32(np.inf), d2)
    return d2.argmin(1)


def kernel(e_actv, e_ap, host):
    e = np.ascontiguousarray(np.asarray(e_actv, dtype=np.float32))
    hostv = np.asarray(host).astype(np.int64)
    in_maps = _prep_inputs(e)
    res = _run(in_maps)

    # Collect marked (i, j) pairs from all cores' row-slice masks.
    ii_l, jj_l = [], []
    for k in range(N_CORES):
        m = res.results[k]["out_mask"]         # [8, 128, 4608] uint8
        cols = _cols_order(k)
        for rs in range(8):
            mu = (m[rs] == 1)
            off = _slice_off(rs)
            if off:
                mu[:, :off] = False            # skipped region: garbage
            pp_, cc = np.nonzero(mu)
            rr = cc + (0 if rs < 4 else (cc >= 4096) * 512)
            ii_l.append(k * BLK + rs * P + pp_)
            jj_l.append(cols[rr])
    ii = np.concatenate(ii_l)
    jj = np.concatenate(jj_l)
    # Drop same-host / self pairs (device doesn't mask them).
    keepp = (hostv[ii] != hostv[jj])
    ii, jj = ii[keepp], jj[keepp]

    # Exact fp32 evaluation of candidates (reference arithmetic), one
    # eval per computed pair; symmetrize afterwards (d2 is symmetric).
    sq32 = (e * e).sum(1, dtype=np.float32)
    g = np.empty(len(ii), dtype=np.float32)
    CH = 2 << 20
    for o in range(0, len(ii), CH):
        sl = slice(o, o + CH)
        g[sl] = np.einsum("nd,nd->n", e[ii[sl]], e[jj[sl]], optimize=True)
    d2c = sq32[ii] + sq32[jj] - 2.0 * np.float32(1.0) * g
    dist = np.sqrt(np.maximum(d2c, 0.0), dtype=np.float32)
    ii, jj = np.concatenate([ii, jj]), np.concatenate([jj, ii])
    dist = np.concatenate([dist, dist])

    # Per-row argmin with first-index tie-break.
    order = np.lexsort((jj, dist, ii))
    oi, oj, od = ii[order], jj[order], dist[order]
    first = np.ones(len(oi), dtype=bool)
    first[1:] = oi[1:] != oi[:-1]
    rows_hit = oi[first]
    idx = np.zeros(N, dtype=np.int64)
    best = np.full(N, np.inf, dtype=np.float64)
    idx[rows_hit] = oj[first]
    best[rows_hit] = od[first].astype(np.float64) ** 2

    # near-tie rows: argmin could be rounding-sensitive -> recompute.
    gap = np.full(N, np.inf)
    pos_first = np.flatnonzero(first)
    pos_second = pos_first + 1
    ok2 = pos_second < len(oi)
    same_row = np.zeros(len(pos_first), dtype=bool)
    same_row[ok2] = oi[pos_second[ok2]] == oi[pos_first[ok2]]
    g2 = np.full(len(pos_first), np.inf)
    g2[same_row] = (od[pos_second[same_row]].astype(np.float64) ** 2
                    - od[pos_first[same_row]].astype(np.float64) ** 2)
    gap[rows_hit] = g2

    rescue = (best > CERT_D2) | (gap < 0.05)
    r_rows = np.flatnonzero(rescue)
    if len(r_rows):
        idx[r_rows] = _exact_rows(e, sq32, hostv, r_rows)

    e_an = np.asarray(e_actv)[idx]
    return (np.asarray(e_actv), np.asarray(e_ap), e_an)


# revision 21
# speedup vs baseline: 1.4407x; 1.3549x over previous
"""Masked nearest-neighbor (AnchorTs2Vec e_an) Trainium2 kernel, v9.

Problem: for e_actv [8192, 256] f32 and host ids [8192], compute
    d2[i,j] = |e_i|^2 + |e_j|^2 - 2 e_i.e_j
    idx[i]  = argmin_{j: host_j != host_i, j != i} d2[i,j]
    e_an    = e_actv[idx]
Returns (e_actv, e_ap, e_an) like the reference.

Device computes a CANDIDATE MASK against a global threshold TAU on an
fp8 DoubleRow approximation of d2 (see v8 notes); host exact-evaluates
marked pairs and rescues uncertified rows. v9 restructures the data
layout to kill the v8 DMA bottleneck (input was 3.58 MB/core of
per-unit duplicated slices; output DMA tail was ~12 us):

CIRCULANT ROW-SLAB LAYOUT. Core k owns rows R_k = [1024k, 1024k+1024)
and computes the (symmetric-unique) pairs of R_k against columns
  C_k = [own block | k+1 | k+2 | k+3 | Z1 | Z2]  (5120 cols)
where Z1/Z2 are complementary 512-col halves of block k+4 chosen so
the 4 quadrants of each d=4 block pair are covered exactly once, and
the diagonal block is upper-triangular via static mask-column offsets.
Units slice two persistent SBUF tiles (L = own rows as lhsT, R = cols
as rhs), so input DMA is the unique data only (~1.6 MB/core), loaded
diag-cols-first so unit 0 starts ~1.5 us in. Per-row thresholds ride
as 64 tail bytes of L (both signs: is_ge for DVE, Sign-bias for ACT).
"""

import numpy as np
import ml_dtypes

import concourse.tile as tile
from concourse import bacc, mybir
from concourse.bass_utils import run_bass_kernel_spmd

N, D = 8192, 256
N_CORES = 8
P = 128
BLK = 1024                  # row/col block per core
RW = 5120                   # rhs unique-column width per core
LW = 1024                   # lhsT unique-row width per core
LT = 32                     # fp8 tail cols of L carrying thresholds
TAU_D2 = 452.0              # global mark threshold on d2
EPS_D2 = 23.0               # device error bound (d2 units)
CERT_D2 = TAU_D2 - 2.0 * EPS_D2

f8 = ml_dtypes.float8_e4m3

# Unit table: (col-chunk offset in R, n rowtiles, diag?) — u0/u1 diag
# halves, u2..u7 blocks k+1..k+3 (A/B row halves), u8 the d=4 quadrants
# (rowtiles 0-3 = A rows x Z1, 4-7 = B rows x Z2), each rt one N=512 MM.
UNITS = ([(0, 4, True), (0, 4, True)]
         + [(c, 4, False) for c in (1024, 1024, 2048, 2048, 3072, 3072)]
         + [(4096, 8, False)])


def _unit_geom(u, rt):
    """Return (row0 in L, col0 in R, width, mask_off, thr_slice)."""
    cblk, nrt, diag = UNITS[u]
    if u < 8:
        row0 = (u % 2) * 512 + rt * P
        col0 = cblk
        w = 1024
        off = (rt * P + (0 if u % 2 == 0 else 512)) if diag else 0
        ts = (u % 2) * 4 + rt
    else:
        row0 = rt * P
        col0 = 4096 + (rt // 4) * 512
        w = 512
        off = 0
        ts = rt
    return row0, col0, w, off, ts


# Greedy DVE/ACT assignment balancing measured per-op cost.
def _mask_engines():
    eng = {}
    # ACT starts with its one-time table-load debt (measured 1283 ns);
    # per-op costs calibrated from v9 HW trace busy times.
    load = {"dve": 0.0, "act": 1283.0}
    for u in range(9):
        for rt in range(UNITS[u][1]):
            _, _, w, off, _ = _unit_geom(u, rt)
            we = w - off
            c_dve = (208 + we) / 0.96
            c_act = (312 + we) / 1.2
            pick = "dve" if load["dve"] + c_dve <= load["act"] + c_act \
                else "act"
            load[pick] += c_dve if pick == "dve" else c_act
            eng[(u, rt)] = pick
    return eng


MASK_ENG = _mask_engines()

_compiled = None


def _build():
    nc = bacc.Bacc("TRN2", target_bir_lowering=False, debug=False,
                   num_devices=N_CORES)
    data_L = nc.dram_tensor("data_L", [P, 2, LW + LT], mybir.dt.float8e4,
                            kind="ExternalInput").ap()
    data_Rd = nc.dram_tensor("data_Rd", [P, 2, 1024], mybir.dt.float8e4,
                             kind="ExternalInput").ap()
    data_Ro = nc.dram_tensor("data_Ro", [P, 2, RW - 1024],
                             mybir.dt.float8e4,
                             kind="ExternalInput").ap()
    out_mask = nc.dram_tensor("out_mask", [9, P, 4096], mybir.dt.uint8,
                              kind="ExternalOutput").ap()

    with tile.TileContext(nc) as tc:
        with tc.tile_pool(name="in", bufs=1) as ip, \
             tc.tile_pool(name="mp", bufs=6) as mp, \
             tc.tile_pool(name="ps", bufs=4, space="PSUM") as ppA:
            tL = ip.tile([P, 2, LW + LT], mybir.dt.float8e4, tag="L")
            tRd = ip.tile([P, 2, 1024], mybir.dt.float8e4, tag="Rd")
            tRo = ip.tile([P, 2, RW - 1024], mybir.dt.float8e4, tag="Ro")
            # need-ordered: L (weights + thresholds), diag cols, then Ro
            # (first needed by unit 2, ~16 matmuls in).
            nc.sync.dma_start(tL, data_L)
            nc.sync.dma_start(tRd, data_Rd)
            nc.sync.dma_start(tRo, data_Ro)
            # thresholds: [128, 8] f32 per sign in the L tails
            thr_dve = tL[:, 0, LW:LW + LT].bitcast(mybir.dt.float32)
            thr_act = tL[:, 1, LW:LW + LT].bitcast(mybir.dt.float32)

            # HAM warm-up: keep the PE busy during the input-DMA wait so
            # the clock gate flips to 8/8 before the real stream begins.
            wtile = ip.tile([P, 2, 512], mybir.dt.float8e4, tag="wu")
            nc.gpsimd.memset(wtile, 0.0)
            wps = ppA.tile([P, 1024], mybir.dt.float32, tag="ps")
            for _ in range(8):
                nc.tensor.matmul(
                    wps[:, 0:512], wtile[:, 0:2, 0:P],
                    wtile[:, 0:2, 0:512],
                    start=True, stop=True,
                    perf_mode=mybir.MatmulPerfMode.DoubleRow)

            for u in range(9):
                nrt = UNITS[u][1]
                mask = mp.tile([P, 4096], mybir.dt.uint8, tag="m")
                for rt in range(nrt):
                    row0, col0, w, off, ts = _unit_geom(u, rt)
                    ps = ppA.tile([P, 1024], mybir.dt.float32, tag="ps")
                    if col0 < 1024:
                        rsrc = tRd
                    else:
                        rsrc = tRo
                        col0 -= 1024
                    for s in range(w // 512):
                        c0 = s * 512
                        nc.tensor.matmul(
                            ps[:, c0:c0 + 512],
                            tL[:, 0:2, row0:row0 + P],
                            rsrc[:, 0:2, col0 + c0:col0 + c0 + 512],
                            start=True, stop=True,
                            perf_mode=mybir.MatmulPerfMode.DoubleRow)
                    mslice = mask[:, rt * w + off:(rt + 1) * w]
                    if MASK_ENG[(u, rt)] == "dve":
                        nc.vector.tensor_scalar(
                            mslice, ps[:, off:w], thr_dve[:, ts:ts + 1],
                            None, op0=mybir.AluOpType.is_ge)
                    else:
                        nc.scalar.activation(
                            mslice, ps[:, off:w],
                            mybir.ActivationFunctionType.Sign,
                            bias=thr_act[:, ts:ts + 1], scale=1.0)
                    eng = nc.gpsimd if u % 2 == 0 else nc.sync
                    if rt == nrt // 2 - 1:
                        eng.dma_start(out_mask[u, :, 0:2048],
                                      mask[:, 0:2048])
                    elif u == 8 and rt == 5:
                        nc.gpsimd.dma_start(out_mask[u, :, 2048:3072],
                                            mask[:, 2048:3072])
                    elif u == 8 and rt == 6:
                        nc.gpsimd.dma_start(out_mask[u, :, 3072:3584],
                                            mask[:, 3072:3584])
                # final chunk on HWDGE (sync): ~0.6us completion latency
                # vs SWDGE ~2us — this chunk is the kernel's tail.
                c0 = 3584 if u == 8 else 2048
                nc.sync.dma_start(out_mask[u, :, c0:4096], mask[:, c0:4096])

    nc.compile()
    return nc


def _cols_order(k):
    """The 5120 global column ids of core k's R tile."""
    parts = [np.arange(k * BLK, (k + 1) * BLK)]
    for d in (1, 2, 3):
        c = ((k + d) % 8) * BLK
        parts.append(np.arange(c, c + BLK))
    m = ((k + 4) % 8) * BLK
    if k < 4:
        z1 = np.arange(m, m + 512)
        z2 = np.arange(m + 512, m + BLK)
    else:
        z1 = np.arange(m + 512, m + BLK)
        z2 = np.arange(m, m + 512)
    parts += [z1, z2]
    return np.concatenate(parts)


def _prep_inputs(e_actv: np.ndarray):
    """Per-core input maps: L (own rows + thr tails), Rd, Ro col tiles.

    Augmented fp8 vectors (K = 256 = 254 data dims + 2 sq slots):
      lhsT rows (i side): [ ek_i (254 dims) ; 1 ; 1 ]
      rhs cols  (j side): [ ek_j (254 dims) ; m1_j ; m2_j ]
    where m1 = fp8(-sq_j/2), m2 = fp8(-sq_j/2 - m1), and ek = e with
    the two smallest-max|e| dims dropped. K index = ck*128 + p.
    """
    e = np.ascontiguousarray(np.asarray(e_actv, dtype=np.float32))
    sq32 = (e * e).sum(1, dtype=np.float32)
    s = sq32.astype(np.float64) / 2.0

    drop = np.argsort(np.abs(e).max(0))[:2]
    keep = np.setdiff1d(np.arange(D), drop)
    ek8T = np.ascontiguousarray(e[:, keep].astype(f8).T)   # [254, 8192]

    m1 = (-s).astype(np.float32).astype(f8)
    m2 = (-s - m1.astype(np.float64)).astype(np.float32).astype(f8)

    aug_l = np.empty((2 * P, N), dtype=f8)   # lhsT side (i): data + 1s
    aug_r = np.empty((2 * P, N), dtype=f8)   # rhs side (j): data + sqs
    aug_l[:254] = ek8T
    aug_l[254] = 1.0
    aug_l[255] = 1.0
    aug_r[:254] = ek8T
    aug_r[254] = m1
    aug_r[255] = m2

    in_maps = []
    for k in range(N_CORES):
        rows = np.arange(k * BLK, (k + 1) * BLK)
        cols = _cols_order(k)
        dL = np.zeros((P, 2, LW + LT), dtype=f8)
        dL[:, 0, :LW] = aug_l[0:P][:, rows]
        dL[:, 1, :LW] = aug_l[P:2 * P][:, rows]
        # thresholds: slice ts covers rows k*1024 + ts*128 + [0:128)
        # DVE: is_ge(ps, t) with t = sq_i/2 - TAU/2
        # ACT: Sign(ps + bias) with bias = -t
        t = (s[rows] - TAU_D2 / 2.0).astype(np.float32).reshape(8, P).T
        dLb = dL.view(np.uint8)
        dLb[:, 0, LW:] = np.ascontiguousarray(t).view(np.uint8)
        dLb[:, 1, LW:] = np.ascontiguousarray(-t).view(np.uint8)
        dR = np.empty((P, 2, RW), dtype=f8)
        dR[:, 0, :] = aug_r[0:P][:, cols]
        dR[:, 1, :] = aug_r[P:2 * P][:, cols]
        in_maps.append({"data_L": dL,
                        "data_Rd": np.ascontiguousarray(dR[:, :, :1024]),
                        "data_Ro": np.ascontiguousarray(dR[:, :, 1024:])})
    return in_maps


def _run(in_maps, trace=False, **kw):
    global _compiled
    if _compiled is None:
        _compiled = _build()
    return run_bass_kernel_spmd(_compiled, in_maps, list(range(N_CORES)),
                                trace=trace, **kw)


def _exact_rows(e, sq32, hostv, rows):
    """Exact fp32 masked argmin for given rows (reference arithmetic)."""
    G = e[rows] @ e.T
    d2 = sq32[rows][:, None] + sq32[None, :] - 2.0 * G
    d2 = np.where(hostv[rows][:, None] == hostv[None, :],
                  np.float32(np.inf), d2)
    return d2.argmin(1)


def kernel(e_actv, e_ap, host):
    e = np.ascontiguousarray(np.asarray(e_actv, dtype=np.float32))
    hostv = np.asarray(host).astype(np.int64)
    in_maps = _prep_inputs(e)
    res = _run(in_maps)

    # Collect marked (i, j) pairs from all cores' unit masks.
    ii_l, jj_l = [], []
    for k in range(N_CORES):
        m = res.results[k]["out_mask"]         # [9, 128, 4096] uint8
        cols = _cols_order(k)
        for u in range(9):
            nrt = UNITS[u][1]
            w = 4096 // nrt
            mu = (m[u] == 1).reshape(P, nrt, w)
            for rt in range(nrt):
                row0, col0, _, off, _ = _unit_geom(u, rt)
                if off:
                    mu[:, rt, :off] = False    # skipped region: garbage
                pp_, ff = np.nonzero(mu[:, rt, :])
                ii_l.append(k * BLK + row0 + pp_)
                jj_l.append(cols[col0 + ff])
    ii = np.concatenate(ii_l)
    jj = np.concatenate(jj_l)
    # Drop same-host / self pairs (device doesn't mask them).
    keepp = (hostv[ii] != hostv[jj])
    ii, jj = ii[keepp], jj[keepp]

    # Exact fp32 evaluation of candidates (reference arithmetic), one
    # eval per computed pair; symmetrize afterwards (d2 is symmetric).
    sq32 = (e * e).sum(1, dtype=np.float32)
    g = np.empty(len(ii), dtype=np.float32)
    CH = 2 << 20
    for o in range(0, len(ii), CH):
        sl = slice(o, o + CH)
        g[sl] = np.einsum("nd,nd->n", e[ii[sl]], e[jj[sl]], optimize=True)
    d2c = sq32[ii] + sq32[jj] - 2.0 * np.float32(1.0) * g
    dist = np.sqrt(np.maximum(d2c, 0.0), dtype=np.float32)
    ii, jj = np.concatenate([ii, jj]), np.concatenate([jj, ii])
    dist = np.concatenate([dist, dist])

    # Per-row argmin with first-index tie-break.
    order = np.lexsort((jj, dist, ii))
    oi, oj, od = ii[order], jj[order], dist[order]
    first = np.ones(len(oi), dtype=bool)
    first[1:] = oi[1:] != oi[:-1]
    rows_hit = oi[first]
    idx = np.zeros(N, dtype=np.int64)
    best = np.full(N, np.inf, dtype=np.float64)
    idx[rows_hit] = oj[first]
    best[rows_hit] = od[first].astype(np.float64) ** 2

    # near-tie rows: argmin could be rounding-sensitive -> recompute.
    gap = np.full(N, np.inf)
    pos_first = np.flatnonzero(first)
    pos_second = pos_first + 1
    ok2 = pos_second < len(oi)
    same_row = np.zeros(len(pos_first), dtype=bool)
    same_row[ok2] = oi[pos_second[ok2]] == oi[pos_first[ok2]]
    g2 = np.full(len(pos_first), np.inf)
    g2[same_row] = (od[pos_second[same_row]].astype(np.float64) ** 2
                    - od[pos_first[same_row]].astype(np.float64) ** 2)
    gap[rows_hit] = g2

    rescue = (best > CERT_D2) | (gap < 0.05)
    r_rows = np.flatnonzero(rescue)
    if len(r_rows):
        idx[r_rows] = _exact_rows(e, sq32, hostv, r_rows)

    e_an = np.asarray(e_actv)[idx]
    return (np.asarray(e_actv), np.asarray(e_ap), e_an)


# revision 22
# speedup vs baseline: 1.4537x; 1.0090x over previous
"""Masked nearest-neighbor (AnchorTs2Vec e_an) Trainium2 kernel, v9.

Problem: for e_actv [8192, 256] f32 and host ids [8192], compute
    d2[i,j] = |e_i|^2 + |e_j|^2 - 2 e_i.e_j
    idx[i]  = argmin_{j: host_j != host_i, j != i} d2[i,j]
    e_an    = e_actv[idx]
Returns (e_actv, e_ap, e_an) like the reference.

Device computes a CANDIDATE MASK against a global threshold TAU on an
fp8 DoubleRow approximation of d2 (see v8 notes); host exact-evaluates
marked pairs and rescues uncertified rows. v9 restructures the data
layout to kill the v8 DMA bottleneck (input was 3.58 MB/core of
per-unit duplicated slices; output DMA tail was ~12 us):

CIRCULANT ROW-SLAB LAYOUT. Core k owns rows R_k = [1024k, 1024k+1024)
and computes the (symmetric-unique) pairs of R_k against columns
  C_k = [own block | k+1 | k+2 | k+3 | Z1 | Z2]  (5120 cols)
where Z1/Z2 are complementary 512-col halves of block k+4 chosen so
the 4 quadrants of each d=4 block pair are covered exactly once, and
the diagonal block is upper-triangular via static mask-column offsets.
Units slice two persistent SBUF tiles (L = own rows as lhsT, R = cols
as rhs), so input DMA is the unique data only (~1.6 MB/core), loaded
diag-cols-first so unit 0 starts ~1.5 us in. Per-row thresholds ride
as 64 tail bytes of L (both signs: is_ge for DVE, Sign-bias for ACT).
"""

import numpy as np
import ml_dtypes

import concourse.tile as tile
from concourse import bacc, mybir
from concourse.bass_utils import run_bass_kernel_spmd

N, D = 8192, 256
N_CORES = 8
P = 128
BLK = 1024                  # row/col block per core
RW = 5120                   # rhs unique-column width per core
LW = 1024                   # lhsT unique-row width per core
LT = 32                     # fp8 tail cols of L carrying thresholds
TAU_D2 = 452.0              # global mark threshold on d2
EPS_D2 = 23.0               # device error bound (d2 units)
CERT_D2 = TAU_D2 - 2.0 * EPS_D2

f8 = ml_dtypes.float8_e4m3

# Unit table: (col-chunk offset in R, n rowtiles, diag?) — u0/u1 diag
# halves, u2..u7 blocks k+1..k+3 (A/B row halves), u8 the d=4 quadrants
# (rowtiles 0-3 = A rows x Z1, 4-7 = B rows x Z2), each rt one N=512 MM.
UNITS = ([(0, 4, True), (0, 4, True)]
         + [(c, 4, False) for c in (1024, 1024, 2048, 2048, 3072, 3072)]
         + [(4096, 8, False)])


def _unit_geom(u, rt):
    """Return (row0 in L, col0 in R, width, mask_off, thr_slice)."""
    cblk, nrt, diag = UNITS[u]
    if u < 8:
        row0 = (u % 2) * 512 + rt * P
        col0 = cblk
        w = 1024
        off = (rt * P + (0 if u % 2 == 0 else 512)) if diag else 0
        ts = (u % 2) * 4 + rt
    else:
        row0 = rt * P
        col0 = 4096 + (rt // 4) * 512
        w = 512
        off = 0
        ts = rt
    return row0, col0, w, off, ts


# Greedy DVE/ACT assignment balancing measured per-op cost.
def _mask_engines():
    eng = {}
    # ACT starts with its one-time table-load debt (measured 1283 ns);
    # per-op costs calibrated from v9 HW trace busy times.
    load = {"dve": 0.0, "act": 1283.0}
    for u in range(9):
        for rt in range(UNITS[u][1]):
            _, _, w, off, _ = _unit_geom(u, rt)
            we = w - off
            c_dve = (208 + we) / 0.96
            c_act = (312 + we) / 1.2
            pick = "dve" if load["dve"] + c_dve <= load["act"] + c_act \
                else "act"
            load[pick] += c_dve if pick == "dve" else c_act
            eng[(u, rt)] = pick
    return eng


MASK_ENG = _mask_engines()

_compiled = None


def _build():
    nc = bacc.Bacc("TRN2", target_bir_lowering=False, debug=False,
                   num_devices=N_CORES)
    data_L = nc.dram_tensor("data_L", [P, 2, LW + LT], mybir.dt.float8e4,
                            kind="ExternalInput").ap()
    data_Rd = nc.dram_tensor("data_Rd", [P, 2, 1024], mybir.dt.float8e4,
                             kind="ExternalInput").ap()
    data_Ro = nc.dram_tensor("data_Ro", [P, 2, RW - 1024],
                             mybir.dt.float8e4,
                             kind="ExternalInput").ap()
    out_mask = nc.dram_tensor("out_mask", [9, P, 4096], mybir.dt.uint8,
                              kind="ExternalOutput").ap()

    with tile.TileContext(nc) as tc:
        with tc.tile_pool(name="in", bufs=1) as ip, \
             tc.tile_pool(name="mp", bufs=6) as mp, \
             tc.tile_pool(name="ps", bufs=4, space="PSUM") as ppA:
            tL = ip.tile([P, 2, LW + LT], mybir.dt.float8e4, tag="L")
            tRd = ip.tile([P, 2, 1024], mybir.dt.float8e4, tag="Rd")
            tRo = ip.tile([P, 2, RW - 1024], mybir.dt.float8e4, tag="Ro")
            # need-ordered: first weights, diag cols, rest of L (incl.
            # thresholds at the tail), then Ro (first needed by unit 2,
            # ~16 matmuls in).
            nc.sync.dma_start(tL[:, :, 0:256], data_L[:, :, 0:256])
            nc.sync.dma_start(tRd, data_Rd)
            nc.sync.dma_start(tL[:, :, 256:LW + LT],
                              data_L[:, :, 256:LW + LT])
            nc.sync.dma_start(tRo, data_Ro)
            # thresholds: [128, 8] f32 per sign in the L tails
            thr_dve = tL[:, 0, LW:LW + LT].bitcast(mybir.dt.float32)
            thr_act = tL[:, 1, LW:LW + LT].bitcast(mybir.dt.float32)

            # HAM warm-up: keep the PE busy during the input-DMA wait so
            # the clock gate flips to 8/8 before the real stream begins.
            wtile = ip.tile([P, 2, 512], mybir.dt.float8e4, tag="wu")
            nc.gpsimd.memset(wtile, 0.0)
            wps = ppA.tile([P, 1024], mybir.dt.float32, tag="ps")
            for _ in range(8):
                nc.tensor.matmul(
                    wps[:, 0:512], wtile[:, 0:2, 0:P],
                    wtile[:, 0:2, 0:512],
                    start=True, stop=True,
                    perf_mode=mybir.MatmulPerfMode.DoubleRow)

            for u in range(9):
                nrt = UNITS[u][1]
                mask = mp.tile([P, 4096], mybir.dt.uint8, tag="m")
                for rt in range(nrt):
                    row0, col0, w, off, ts = _unit_geom(u, rt)
                    ps = ppA.tile([P, 1024], mybir.dt.float32, tag="ps")
                    if col0 < 1024:
                        rsrc = tRd
                    else:
                        rsrc = tRo
                        col0 -= 1024
                    for s in range(w // 512):
                        c0 = s * 512
                        nc.tensor.matmul(
                            ps[:, c0:c0 + 512],
                            tL[:, 0:2, row0:row0 + P],
                            rsrc[:, 0:2, col0 + c0:col0 + c0 + 512],
                            start=True, stop=True,
                            perf_mode=mybir.MatmulPerfMode.DoubleRow)
                    mslice = mask[:, rt * w + off:(rt + 1) * w]
                    if MASK_ENG[(u, rt)] == "dve":
                        nc.vector.tensor_scalar(
                            mslice, ps[:, off:w], thr_dve[:, ts:ts + 1],
                            None, op0=mybir.AluOpType.is_ge)
                    else:
                        nc.scalar.activation(
                            mslice, ps[:, off:w],
                            mybir.ActivationFunctionType.Sign,
                            bias=thr_act[:, ts:ts + 1], scale=1.0)
                    eng = nc.gpsimd if u % 2 == 0 else nc.sync
                    if rt == nrt // 2 - 1:
                        eng.dma_start(out_mask[u, :, 0:2048],
                                      mask[:, 0:2048])
                    elif u == 8 and rt == 5:
                        nc.gpsimd.dma_start(out_mask[u, :, 2048:3072],
                                            mask[:, 2048:3072])
                    elif u == 8 and rt == 6:
                        nc.gpsimd.dma_start(out_mask[u, :, 3072:3584],
                                            mask[:, 3072:3584])
                # final chunk on HWDGE (sync): ~0.6us completion latency
                # vs SWDGE ~2us — this chunk is the kernel's tail.
                c0 = 3584 if u == 8 else 2048
                nc.sync.dma_start(out_mask[u, :, c0:4096], mask[:, c0:4096])

    nc.compile()
    return nc


def _cols_order(k):
    """The 5120 global column ids of core k's R tile."""
    parts = [np.arange(k * BLK, (k + 1) * BLK)]
    for d in (1, 2, 3):
        c = ((k + d) % 8) * BLK
        parts.append(np.arange(c, c + BLK))
    m = ((k + 4) % 8) * BLK
    if k < 4:
        z1 = np.arange(m, m + 512)
        z2 = np.arange(m + 512, m + BLK)
    else:
        z1 = np.arange(m + 512, m + BLK)
        z2 = np.arange(m, m + 512)
    parts += [z1, z2]
    return np.concatenate(parts)


def _prep_inputs(e_actv: np.ndarray):
    """Per-core input maps: L (own rows + thr tails), Rd, Ro col tiles.

    Augmented fp8 vectors (K = 256 = 254 data dims + 2 sq slots):
      lhsT rows (i side): [ ek_i (254 dims) ; 1 ; 1 ]
      rhs cols  (j side): [ ek_j (254 dims) ; m1_j ; m2_j ]
    where m1 = fp8(-sq_j/2), m2 = fp8(-sq_j/2 - m1), and ek = e with
    the two smallest-max|e| dims dropped. K index = ck*128 + p.
    """
    e = np.ascontiguousarray(np.asarray(e_actv, dtype=np.float32))
    sq32 = (e * e).sum(1, dtype=np.float32)
    s = sq32.astype(np.float64) / 2.0

    drop = np.argsort(np.abs(e).max(0))[:2]
    keep = np.setdiff1d(np.arange(D), drop)
    ek8T = np.ascontiguousarray(e[:, keep].astype(f8).T)   # [254, 8192]

    m1 = (-s).astype(np.float32).astype(f8)
    m2 = (-s - m1.astype(np.float64)).astype(np.float32).astype(f8)

    aug_l = np.empty((2 * P, N), dtype=f8)   # lhsT side (i): data + 1s
    aug_r = np.empty((2 * P, N), dtype=f8)   # rhs side (j): data + sqs
    aug_l[:254] = ek8T
    aug_l[254] = 1.0
    aug_l[255] = 1.0
    aug_r[:254] = ek8T
    aug_r[254] = m1
    aug_r[255] = m2

    in_maps = []
    for k in range(N_CORES):
        rows = np.arange(k * BLK, (k + 1) * BLK)
        cols = _cols_order(k)
        dL = np.zeros((P, 2, LW + LT), dtype=f8)
        dL[:, 0, :LW] = aug_l[0:P][:, rows]
        dL[:, 1, :LW] = aug_l[P:2 * P][:, rows]
        # thresholds: slice ts covers rows k*1024 + ts*128 + [0:128)
        # DVE: is_ge(ps, t) with t = sq_i/2 - TAU/2
        # ACT: Sign(ps + bias) with bias = -t
        t = (s[rows] - TAU_D2 / 2.0).astype(np.float32).reshape(8, P).T
        dLb = dL.view(np.uint8)
        dLb[:, 0, LW:] = np.ascontiguousarray(t).view(np.uint8)
        dLb[:, 1, LW:] = np.ascontiguousarray(-t).view(np.uint8)
        dR = np.empty((P, 2, RW), dtype=f8)
        dR[:, 0, :] = aug_r[0:P][:, cols]
        dR[:, 1, :] = aug_r[P:2 * P][:, cols]
        in_maps.append({"data_L": dL,
                        "data_Rd": np.ascontiguousarray(dR[:, :, :1024]),
                        "data_Ro": np.ascontiguousarray(dR[:, :, 1024:])})
    return in_maps


def _run(in_maps, trace=False, **kw):
    global _compiled
    if _compiled is None:
        _compiled = _build()
    return run_bass_kernel_spmd(_compiled, in_maps, list(range(N_CORES)),
                                trace=trace, **kw)


def _exact_rows(e, sq32, hostv, rows):
    """Exact fp32 masked argmin for given rows (reference arithmetic)."""
    G = e[rows] @ e.T
    d2 = sq32[rows][:, None] + sq32[None, :] - 2.0 * G
    d2 = np.where(hostv[rows][:, None] == hostv[None, :],
                  np.float32(np.inf), d2)
    return d2.argmin(1)


def kernel(e_actv, e_ap, host):
    e = np.ascontiguousarray(np.asarray(e_actv, dtype=np.float32))
    hostv = np.asarray(host).astype(np.int64)
    in_maps = _prep_inputs(e)
    res = _run(in_maps)

    # Collect marked (i, j) pairs from all cores' unit masks.
    ii_l, jj_l = [], []
    for k in range(N_CORES):
        m = res.results[k]["out_mask"]         # [9, 128, 4096] uint8
        cols = _cols_order(k)
        for u in range(9):
            nrt = UNITS[u][1]
            w = 4096 // nrt
            mu = (m[u] == 1).reshape(P, nrt, w)
            for rt in range(nrt):
                row0, col0, _, off, _ = _unit_geom(u, rt)
                if off:
                    mu[:, rt, :off] = False    # skipped region: garbage
                pp_, ff = np.nonzero(mu[:, rt, :])
                ii_l.append(k * BLK + row0 + pp_)
                jj_l.append(cols[col0 + ff])
    ii = np.concatenate(ii_l)
    jj = np.concatenate(jj_l)
    # Drop same-host / self pairs (device doesn't mask them).
    keepp = (hostv[ii] != hostv[jj])
    ii, jj = ii[keepp], jj[keepp]

    # Exact fp32 evaluation of candidates (reference arithmetic), one
    # eval per computed pair; symmetrize afterwards (d2 is symmetric).
    sq32 = (e * e).sum(1, dtype=np.float32)
    g = np.empty(len(ii), dtype=np.float32)
    CH = 2 << 20
    for o in range(0, len(ii), CH):
        sl = slice(o, o + CH)
        g[sl] = np.einsum("nd,nd->n", e[ii[sl]], e[jj[sl]], optimize=True)
    d2c = sq32[ii] + sq32[jj] - 2.0 * np.float32(1.0) * g
    dist = np.sqrt(np.maximum(d2c, 0.0), dtype=np.float32)
    ii, jj = np.concatenate([ii, jj]), np.concatenate([jj, ii])
    dist = np.concatenate([dist, dist])

    # Per-row argmin with first-index tie-break.
    order = np.lexsort((jj, dist, ii))
    oi, oj, od = ii[order], jj[order], dist[order]
    first = np.ones(len(oi), dtype=bool)
    first[1:] = oi[1:] != oi[:-1]
    rows_hit = oi[first]
    idx = np.zeros(N, dtype=np.int64)
    best = np.full(N, np.inf, dtype=np.float64)
    idx[rows_hit] = oj[first]
    best[rows_hit] = od[first].astype(np.float64) ** 2

    # near-tie rows: argmin could be rounding-sensitive -> recompute.
    gap = np.full(N, np.inf)
    pos_first = np.flatnonzero(first)
    pos_second = pos_first + 1
    ok2 = pos_second < len(oi)
    same_row = np.zeros(len(pos_first), dtype=bool)
    same_row[ok2] = oi[pos_second[ok2]] == oi[pos_first[ok2]]
    g2 = np.full(len(pos_first), np.inf)
    g2[same_row] = (od[pos_second[same_row]].astype(np.float64) ** 2
                    - od[pos_first[same_row]].astype(np.float64) ** 2)
    gap[rows_hit] = g2

    rescue = (best > CERT_D2) | (gap < 0.05)
    r_rows = np.flatnonzero(rescue)
    if len(r_rows):
        idx[r_rows] = _exact_rows(e, sq32, hostv, r_rows)

    e_an = np.asarray(e_actv)[idx]
    return (np.asarray(e_actv), np.asarray(e_ap), e_an)


# revision 24
# speedup vs baseline: 1.4734x; 1.0135x over previous
"""Masked nearest-neighbor (AnchorTs2Vec e_an) Trainium2 kernel, v15.

Problem: for e_actv [8192, 256] f32 and host ids [8192], compute
    d2[i,j] = |e_i|^2 + |e_j|^2 - 2 e_i.e_j
    idx[i]  = argmin_{j: host_j != host_i, j != i} d2[i,j]
    e_an    = e_actv[idx]
Returns (e_actv, e_ap, e_an) like the reference.

Device computes a CANDIDATE MASK against a global threshold TAU on an
fp8 DoubleRow approximation of d2 (see v8 notes); host exact-evaluates
marked pairs and rescues uncertified rows.

CIRCULANT ROW-SLAB LAYOUT (v9): core k owns rows R_k = [1024k,
1024k+1024) and computes the symmetric-unique pairs of R_k against
columns C_k = [own block | k+1 | k+2 | k+3 | Z1 | Z2] (5120 cols),
where Z1/Z2 are complementary 512-col halves of block k+4 chosen so
the 4 quadrants of each d=4 block pair are covered exactly once, and
the diagonal block is upper-triangular via static mask-column offsets.
Units slice two persistent SBUF tiles (L = own rows as lhsT, R = cols
as rhs), so input DMA is the unique data only (~1.6 MB/core vs 3.58
duplicated in v8). Per-row thresholds ride as 64 tail bytes of L
(both signs: is_ge for DVE, Sign-bias for ACT).

Schedule (v10-v15, HW-trace-driven):
- 8 dummy N=512 matmuls on a memset scratch tile run during the input
  DMA wait, so the PE's HAM clock gate flips to 8/8 (2.4 GHz) before
  the real stream starts (saves ~5 us of cold matmuls).
- PSUM pool: uniform [128,1024] tiles x 4 bufs = all 8 banks; the
  4-deep ring keeps matmuls ~2 groups ahead of the DVE/ACT mask ops
  (2-deep rings serialize the ring and HAM-cool the PE: +15 us).
- Mask ops balanced DVE/ACT by measured cost; both engines read PSUM
  at 32b/cycle, which is the structural floor (~33.3k elems/partition
  across 2 engines ~= 15.4 us + per-op overhead).
- Input DMAs need-ordered (first weights, diag cols, L rest, Ro);
  output masks leave per unit in 2-4 chunks on alternating SWDGE/
  HWDGE queues, final chunk on HWDGE (lower completion latency).
Fixed framework cost (preamble barriers, 253-semaphore reset storm in
the bass_exec epilogue, ~7+6 us) dominates the remaining gap.
"""

import numpy as np
import ml_dtypes

import concourse.tile as tile
from concourse import bacc, mybir
from concourse.bass_utils import run_bass_kernel_spmd

N, D = 8192, 256
N_CORES = 8
P = 128
BLK = 1024                  # row/col block per core
RW = 5120                   # rhs unique-column width per core
LW = 1024                   # lhsT unique-row width per core
LT = 32                     # fp8 tail cols of L carrying thresholds
TAU_D2 = 452.0              # global mark threshold on d2
EPS_D2 = 23.0               # device error bound (d2 units)
CERT_D2 = TAU_D2 - 2.0 * EPS_D2

f8 = ml_dtypes.float8_e4m3

# Unit table: (col-chunk offset in R, n rowtiles, diag?) — u0/u1 diag
# halves, u2..u7 blocks k+1..k+3 (A/B row halves), u8 the d=4 quadrants
# (rowtiles 0-3 = A rows x Z1, 4-7 = B rows x Z2), each rt one N=512 MM.
UNITS = ([(0, 4, True), (0, 4, True)]
         + [(c, 4, False) for c in (1024, 1024, 2048, 2048, 3072, 3072)]
         + [(4096, 8, False)])


def _unit_geom(u, rt):
    """Return (row0 in L, col0 in R, width, mask_off, thr_slice)."""
    cblk, nrt, diag = UNITS[u]
    if u < 8:
        row0 = (u % 2) * 512 + rt * P
        col0 = cblk
        w = 1024
        off = (rt * P + (0 if u % 2 == 0 else 512)) if diag else 0
        ts = (u % 2) * 4 + rt
    else:
        row0 = rt * P
        col0 = 4096 + (rt // 4) * 512
        w = 512
        off = 0
        ts = rt
    return row0, col0, w, off, ts


# Greedy DVE/ACT assignment balancing measured per-op cost.
def _mask_engines():
    eng = {}
    # ACT starts with its one-time table-load debt (measured 1283 ns);
    # per-op costs calibrated from v9 HW trace busy times.
    load = {"dve": 0.0, "act": 1283.0}
    for u in range(9):
        for rt in range(UNITS[u][1]):
            _, _, w, off, _ = _unit_geom(u, rt)
            we = w - off
            c_dve = (100 + we) / 0.96
            c_act = (245 + we) / 1.2
            pick = "dve" if load["dve"] + c_dve <= load["act"] + c_act \
                else "act"
            load[pick] += c_dve if pick == "dve" else c_act
            eng[(u, rt)] = pick
    return eng


MASK_ENG = _mask_engines()

_compiled = None


def _build():
    nc = bacc.Bacc("TRN2", target_bir_lowering=False, debug=False,
                   num_devices=N_CORES)
    data_L = nc.dram_tensor("data_L", [P, 2, LW + LT], mybir.dt.float8e4,
                            kind="ExternalInput").ap()
    data_Rd = nc.dram_tensor("data_Rd", [P, 2, 1024], mybir.dt.float8e4,
                             kind="ExternalInput").ap()
    data_Ro = nc.dram_tensor("data_Ro", [P, 2, RW - 1024],
                             mybir.dt.float8e4,
                             kind="ExternalInput").ap()
    out_mask = nc.dram_tensor("out_mask", [9, P, 4096], mybir.dt.uint8,
                              kind="ExternalOutput").ap()

    with tile.TileContext(nc) as tc:
        with tc.tile_pool(name="in", bufs=1) as ip, \
             tc.tile_pool(name="mp", bufs=6) as mp, \
             tc.tile_pool(name="ps", bufs=4, space="PSUM") as ppA:
            tL = ip.tile([P, 2, LW + LT], mybir.dt.float8e4, tag="L")
            tRd = ip.tile([P, 2, 1024], mybir.dt.float8e4, tag="Rd")
            tRo = ip.tile([P, 2, RW - 1024], mybir.dt.float8e4, tag="Ro")
            # need-ordered: first weights, diag cols, rest of L (incl.
            # thresholds at the tail), then Ro (first needed by unit 2,
            # ~16 matmuls in).
            nc.sync.dma_start(tL[:, :, 0:256], data_L[:, :, 0:256])
            nc.sync.dma_start(tRd, data_Rd)
            nc.sync.dma_start(tL[:, :, 256:LW + LT],
                              data_L[:, :, 256:LW + LT])
            nc.sync.dma_start(tRo, data_Ro)
            # thresholds: [128, 8] f32 per sign in the L tails
            thr_dve = tL[:, 0, LW:LW + LT].bitcast(mybir.dt.float32)
            thr_act = tL[:, 1, LW:LW + LT].bitcast(mybir.dt.float32)

            # HAM warm-up: keep the PE busy during the input-DMA wait so
            # the clock gate flips to 8/8 before the real stream begins.
            wtile = ip.tile([P, 2, 512], mybir.dt.float8e4, tag="wu")
            nc.gpsimd.memset(wtile, 0.0)
            wps = ppA.tile([P, 1024], mybir.dt.float32, tag="ps")
            for _ in range(8):
                nc.tensor.matmul(
                    wps[:, 0:512], wtile[:, 0:2, 0:P],
                    wtile[:, 0:2, 0:512],
                    start=True, stop=True,
                    perf_mode=mybir.MatmulPerfMode.DoubleRow)

            for u in range(9):
                nrt = UNITS[u][1]
                mask = mp.tile([P, 4096], mybir.dt.uint8, tag="m")
                for rt in range(nrt):
                    row0, col0, w, off, ts = _unit_geom(u, rt)
                    ps = ppA.tile([P, 1024], mybir.dt.float32, tag="ps")
                    if col0 < 1024:
                        rsrc = tRd
                    else:
                        rsrc = tRo
                        col0 -= 1024
                    for s in range(w // 512):
                        c0 = s * 512
                        nc.tensor.matmul(
                            ps[:, c0:c0 + 512],
                            tL[:, 0:2, row0:row0 + P],
                            rsrc[:, 0:2, col0 + c0:col0 + c0 + 512],
                            start=True, stop=True,
                            perf_mode=mybir.MatmulPerfMode.DoubleRow)
                    mslice = mask[:, rt * w + off:(rt + 1) * w]
                    if MASK_ENG[(u, rt)] == "dve":
                        nc.vector.tensor_scalar(
                            mslice, ps[:, off:w], thr_dve[:, ts:ts + 1],
                            None, op0=mybir.AluOpType.is_ge)
                    else:
                        nc.scalar.activation(
                            mslice, ps[:, off:w],
                            mybir.ActivationFunctionType.Sign,
                            bias=thr_act[:, ts:ts + 1], scale=1.0)
                    eng = nc.gpsimd if u % 2 == 0 else nc.sync
                    if rt == nrt // 2 - 1:
                        eng.dma_start(out_mask[u, :, 0:2048],
                                      mask[:, 0:2048])
                    elif u == 8 and rt == 5:
                        nc.gpsimd.dma_start(out_mask[u, :, 2048:3072],
                                            mask[:, 2048:3072])
                    elif u == 8 and rt == 6:
                        nc.gpsimd.dma_start(out_mask[u, :, 3072:3584],
                                            mask[:, 3072:3584])
                # final chunk on HWDGE (sync): ~0.6us completion latency
                # vs SWDGE ~2us — this chunk is the kernel's tail.
                c0 = 3584 if u == 8 else 2048
                nc.sync.dma_start(out_mask[u, :, c0:4096], mask[:, c0:4096])

    nc.compile()
    return nc


def _cols_order(k):
    """The 5120 global column ids of core k's R tile."""
    parts = [np.arange(k * BLK, (k + 1) * BLK)]
    for d in (1, 2, 3):
        c = ((k + d) % 8) * BLK
        parts.append(np.arange(c, c + BLK))
    m = ((k + 4) % 8) * BLK
    if k < 4:
        z1 = np.arange(m, m + 512)
        z2 = np.arange(m + 512, m + BLK)
    else:
        z1 = np.arange(m + 512, m + BLK)
        z2 = np.arange(m, m + 512)
    parts += [z1, z2]
    return np.concatenate(parts)


def _prep_inputs(e_actv: np.ndarray):
    """Per-core input maps: L (own rows + thr tails), Rd, Ro col tiles.

    Augmented fp8 vectors (K = 256 = 254 data dims + 2 sq slots):
      lhsT rows (i side): [ ek_i (254 dims) ; 1 ; 1 ]
      rhs cols  (j side): [ ek_j (254 dims) ; m1_j ; m2_j ]
    where m1 = fp8(-sq_j/2), m2 = fp8(-sq_j/2 - m1), and ek = e with
    the two smallest-max|e| dims dropped. K index = ck*128 + p.
    """
    e = np.ascontiguousarray(np.asarray(e_actv, dtype=np.float32))
    sq32 = (e * e).sum(1, dtype=np.float32)
    s = sq32.astype(np.float64) / 2.0

    drop = np.argsort(np.abs(e).max(0))[:2]
    keep = np.setdiff1d(np.arange(D), drop)
    ek8T = np.ascontiguousarray(e[:, keep].astype(f8).T)   # [254, 8192]

    m1 = (-s).astype(np.float32).astype(f8)
    m2 = (-s - m1.astype(np.float64)).astype(np.float32).astype(f8)

    aug_l = np.empty((2 * P, N), dtype=f8)   # lhsT side (i): data + 1s
    aug_r = np.empty((2 * P, N), dtype=f8)   # rhs side (j): data + sqs
    aug_l[:254] = ek8T
    aug_l[254] = 1.0
    aug_l[255] = 1.0
    aug_r[:254] = ek8T
    aug_r[254] = m1
    aug_r[255] = m2

    in_maps = []
    for k in range(N_CORES):
        rows = np.arange(k * BLK, (k + 1) * BLK)
        cols = _cols_order(k)
        dL = np.zeros((P, 2, LW + LT), dtype=f8)
        dL[:, 0, :LW] = aug_l[0:P][:, rows]
        dL[:, 1, :LW] = aug_l[P:2 * P][:, rows]
        # thresholds: slice ts covers rows k*1024 + ts*128 + [0:128)
        # DVE: is_ge(ps, t) with t = sq_i/2 - TAU/2
        # ACT: Sign(ps + bias) with bias = -t
        t = (s[rows] - TAU_D2 / 2.0).astype(np.float32).reshape(8, P).T
        dLb = dL.view(np.uint8)
        dLb[:, 0, LW:] = np.ascontiguousarray(t).view(np.uint8)
        dLb[:, 1, LW:] = np.ascontiguousarray(-t).view(np.uint8)
        dR = np.empty((P, 2, RW), dtype=f8)
        dR[:, 0, :] = aug_r[0:P][:, cols]
        dR[:, 1, :] = aug_r[P:2 * P][:, cols]
        in_maps.append({"data_L": dL,
                        "data_Rd": np.ascontiguousarray(dR[:, :, :1024]),
                        "data_Ro": np.ascontiguousarray(dR[:, :, 1024:])})
    return in_maps


def _run(in_maps, trace=False, **kw):
    global _compiled
    if _compiled is None:
        _compiled = _build()
    return run_bass_kernel_spmd(_compiled, in_maps, list(range(N_CORES)),
                                trace=trace, **kw)


def _exact_rows(e, sq32, hostv, rows):
    """Exact fp32 masked argmin for given rows (reference arithmetic)."""
    G = e[rows] @ e.T
    d2 = sq32[rows][:, None] + sq32[None, :] - 2.0 * G
    d2 = np.where(hostv[rows][:, None] == hostv[None, :],
                  np.float32(np.inf), d2)
    return d2.argmin(1)


def kernel(e_actv, e_ap, host):
    e = np.ascontiguousarray(np.asarray(e_actv, dtype=np.float32))
    hostv = np.asarray(host).astype(np.int64)
    in_maps = _prep_inputs(e)
    res = _run(in_maps)

    # Collect marked (i, j) pairs from all cores' unit masks.
    ii_l, jj_l = [], []
    for k in range(N_CORES):
        m = res.results[k]["out_mask"]         # [9, 128, 4096] uint8
        cols = _cols_order(k)
        for u in range(9):
            nrt = UNITS[u][1]
            w = 4096 // nrt
            mu = (m[u] == 1).reshape(P, nrt, w)
            for rt in range(nrt):
                row0, col0, _, off, _ = _unit_geom(u, rt)
                if off:
                    mu[:, rt, :off] = False    # skipped region: garbage
                pp_, ff = np.nonzero(mu[:, rt, :])
                ii_l.append(k * BLK + row0 + pp_)
                jj_l.append(cols[col0 + ff])
    ii = np.concatenate(ii_l)
    jj = np.concatenate(jj_l)
    # Drop same-host / self pairs (device doesn't mask them).
    keepp = (hostv[ii] != hostv[jj])
    ii, jj = ii[keepp], jj[keepp]

    # Exact fp32 evaluation of candidates (reference arithmetic), one
    # eval per computed pair; symmetrize afterwards (d2 is symmetric).
    sq32 = (e * e).sum(1, dtype=np.float32)
    g = np.empty(len(ii), dtype=np.float32)
    CH = 2 << 20
    for o in range(0, len(ii), CH):
        sl = slice(o, o + CH)
        g[sl] = np.einsum("nd,nd->n", e[ii[sl]], e[jj[sl]], optimize=True)
    d2c = sq32[ii] + sq32[jj] - 2.0 * np.float32(1.0) * g
    dist = np.sqrt(np.maximum(d2c, 0.0), dtype=np.float32)
    ii, jj = np.concatenate([ii, jj]), np.concatenate([jj, ii])
    dist = np.concatenate([dist, dist])

    # Per-row argmin with first-index tie-break.
    order = np.lexsort((jj, dist, ii))
    oi, oj, od = ii[order], jj[order], dist[order]
    first = np.ones(len(oi), dtype=bool)
    first[1:] = oi[1:] != oi[:-1]
    rows_hit = oi[first]
    idx = np.zeros(N, dtype=np.int64)
    best = np.full(N, np.inf, dtype=np.float64)
    idx[rows_hit] = oj[first]
    best[rows_hit] = od[first].astype(np.float64) ** 2

    # near-tie rows: argmin could be rounding-sensitive -> recompute.
    gap = np.full(N, np.inf)
    pos_first = np.flatnonzero(first)
    pos_second = pos_first + 1
    ok2 = pos_second < len(oi)
    same_row = np.zeros(len(pos_first), dtype=bool)
    same_row[ok2] = oi[pos_second[ok2]] == oi[pos_first[ok2]]
    g2 = np.full(len(pos_first), np.inf)
    g2[same_row] = (od[pos_second[same_row]].astype(np.float64) ** 2
                    - od[pos_first[same_row]].astype(np.float64) ** 2)
    gap[rows_hit] = g2

    rescue = (best > CERT_D2) | (gap < 0.05)
    r_rows = np.flatnonzero(rescue)
    if len(r_rows):
        idx[r_rows] = _exact_rows(e, sq32, hostv, r_rows)

    e_an = np.asarray(e_actv)[idx]
    return (np.asarray(e_actv), np.asarray(e_ap), e_an)
